# revision 1
# baseline (speedup 1.0000x reference)
"""DIN-attention kernel for Trainium2, 8-core SPMD.

Reference computation (per batch b, seq pos l, x = item_seq[b, l]):
    mlp_in = [tgt, x, x-tgt, x*tgt]           (4D = 512)
    h      = relu(mlp_in @ W1 + b1)           (2D = 256)
    score  = h @ W2 + b2                      (1)
    out_b  = sum_l score[l] * x[l] * (l < seq_len[b])

Algebraic restructure (W1 = [A; B; C; Dm] in 128-row blocks):
    z   = x @ (B + C) + (x*tgt) @ Dm + c_b,   c_b = tgt_b @ (A - C) + b1
    out = sum_{l < n_b} (W2.T relu(z) + b2) * x[l]

Device strategy (per core):
  - Batches sorted by seq_len descending; slot s holds global ranks
    [8s, 8s+8), one per core, padded to a shared per-slot length L_s
    (max over the 8, rounded even).  Zero-padded columns contribute
    exactly 0 to the output, so padding is safe, and all 8 cores run an
    identical (SPMD) program while loading only ~half the dense bytes.
  - Tokens packed host-side into a transposed (128=D, T) fp32 array per
    core; compute in the "hidden-on-partitions" layout:
      zT (128=hid_half, n) = Wbc_h.T @ X + Wd_h.T @ Y + Cwin_h.T @ IND
    with Y = X * tgt_col (per-slot, gpsimd) and IND a host-built 0/1
    (32, T) slot-window indicator; Cwin packs the c_b bias rows.
  - relu on ScalarE, then score broadcast to all 128 partitions in one
    PSUM accumulation: P = W2rep0.T @ r0 + W2rep1.T @ r1 + b2 * ones
    (W2rep[k, m] = W2[k] for every m, so every output row = score row).
  - Final per-slot reduce: fused DVE tensor_tensor_reduce
    acc[:, s] = sum_cols (X * P), chained across 512-tiles via initial.
  - Matmuls run in float32r (fp32 storage, single-pass PE streaming).
"""

import sys

import numpy as np

for _p in ("/opt/trn_rl_repo",):
    if _p not in sys.path:
        sys.path.insert(0, _p)

import concourse.bacc as bacc
import concourse.bass as bass
import concourse.tile as tile
from concourse import mybir
from concourse.bass_utils import run_bass_kernel_spmd

assert bass  # re-exported for callers

B_FULL = 2048
L_FULL = 200
D = 128
N_CORES = 8
HID = 256  # 2D
TILE_N = 512  # fp32 PSUM bank columns
CHUNK_TARGET = 8192  # tokens per streamed chunk (slot-aligned)
F32 = mybir.dt.float32
F32R = mybir.dt.float32r
BF16 = mybir.dt.bfloat16

HOST_Y_BF16 = True  # ship Y = X*tgt as a host-packed bf16 array
RELU_BF16 = False  # bf16 ACT output is broken on TRN2 HW (probe E); use f32r
REDUCE_MODE = "stt"  # "ttr" | "stt" | "ttred"  (final per-slot reduce impl)
XB_BF16 = True  # ship X itself in bf16 (halves X DMA; bf16 h-matmuls)
FIRST_CHUNK = 4096  # smaller first chunk to hide pipeline fill
STREAM_BUFS = 2  # chunk-level double buffering
RB_BUFS = 2  # relu/dump tile buffering
B2VAL = [0.0]  # b2 constant, set by build_all before tracing


def _plan(seq_len):
    """Slot plan shared by all cores (SPMD: identical program)."""
    n = np.clip(np.asarray(seq_len).astype(np.int64), 0, L_FULL)
    order = np.argsort(-n, kind="stable")  # descending
    n_sorted = n[order]
    slot_lens = []
    for s in range(B_FULL // N_CORES):
        m = int(n_sorted[N_CORES * s])  # max of ranks [8s, 8s+8)
        if m <= 0:
            break
        slot_lens.append(m + (m & 1))  # round up to even
    S = len(slot_lens)
    offs = np.zeros(S + 1, dtype=np.int64)
    offs[1:] = np.cumsum(slot_lens)
    T = int(offs[-1])

    # chunks: contiguous slot ranges with <= CHUNK_TARGET tokens.
    # The first chunk is smaller so compute starts before the bulk DMA.
    chunks = []  # (slot_a, slot_b, tok_off, tok_len)
    sa = 0
    while sa < S:
        cap = FIRST_CHUNK if not chunks else CHUNK_TARGET
        sb = sa
        while sb < S and offs[sb + 1] - offs[sa] <= cap:
            sb += 1
        if sb == sa:
            sb = sa + 1
        chunks.append((sa, sb, int(offs[sa]), int(offs[sb] - offs[sa])))
        sa = sb
    return n, order, slot_lens, offs, T, chunks


def _build_program(slot_lens, offs, T, chunks):
    S = len(slot_lens)
    NW = (S + 31) // 32  # 32-slot bias windows
    nc = bacc.Bacc("TRN2", target_bir_lowering=False, debug=False)

    RDT = BF16 if RELU_BF16 else F32R
    YDT = BF16 if HOST_Y_BF16 else F32
    XDT = BF16 if XB_BF16 else F32R

    xt_d = nc.dram_tensor("xt", [D, T], XDT, kind="ExternalInput")
    ind_d = nc.dram_tensor("ind", [32, T], BF16, kind="ExternalInput")
    if HOST_Y_BF16:
        yb_d = nc.dram_tensor("yb", [D, T], BF16, kind="ExternalInput")
    else:
        tgt_d = nc.dram_tensor("tgt", [D, S], F32, kind="ExternalInput")
    cbw_d = nc.dram_tensor("cbw", [32, NW * HID], BF16, kind="ExternalInput")
    wbc_d = nc.dram_tensor("wbc", [D, HID], XDT, kind="ExternalInput")
    wd_d = nc.dram_tensor("wd", [D, HID], YDT, kind="ExternalInput")
    w2r_d = nc.dram_tensor("w2r", [D, HID], RDT, kind="ExternalInput")
    out_d = nc.dram_tensor("out_t", [D, 256], F32, kind="ExternalOutput")

    cmax = max(c[3] for c in chunks)

    with tile.TileContext(nc) as tc:
        with (
            tc.tile_pool(name="const", bufs=1) as cpool,
            tc.tile_pool(name="xst", bufs=STREAM_BUFS) as xpool,
            tc.tile_pool(name="yst", bufs=STREAM_BUFS) as ypool,
            tc.tile_pool(name="ist", bufs=STREAM_BUFS) as ipool,
            tc.tile_pool(name="rst", bufs=RB_BUFS) as rpool,
            tc.tile_pool(name="dst", bufs=RB_BUFS) as dpool,
            tc.tile_pool(name="ps", bufs=2, space="PSUM") as pspool,
        ):
            wbc = cpool.tile([D, HID], XDT, tag="wbc")
            wd = cpool.tile([D, HID], YDT, tag="wd")
            w2r = cpool.tile([D, HID], RDT, tag="w2r")
            cbw = cpool.tile([32, NW * HID], BF16, tag="cbw")
            acc = cpool.tile([D, 256], F32, tag="acc")
            aux = cpool.tile([D, 2], F32, tag="aux")

            nc.sync.dma_start(out=wbc[:], in_=wbc_d[:])
            nc.sync.dma_start(out=wd[:], in_=wd_d[:])
            nc.sync.dma_start(out=w2r[:], in_=w2r_d[:])
            if not HOST_Y_BF16:
                tgt = cpool.tile([D, S], F32, tag="tgt")
                nc.sync.dma_start(out=tgt[:], in_=tgt_d[:])
            nc.sync.dma_start(out=cbw[:], in_=cbw_d[:])
            nc.vector.memset(acc[:], 0.0)

            for sa, sb, toff, tlen in chunks:
                x = xpool.tile([D, cmax], XDT, tag="x")
                y = ypool.tile([D, cmax], YDT, tag="y")
                indt = ipool.tile([32, cmax], BF16, tag="ind")
                nc.sync.dma_start(out=x[:, :tlen], in_=xt_d[:, toff : toff + tlen])
                nc.sync.dma_start(out=indt[:, :tlen], in_=ind_d[:, toff : toff + tlen])

                if HOST_Y_BF16:
                    nc.sync.dma_start(
                        out=y[:, :tlen], in_=yb_d[:, toff : toff + tlen]
                    )
                else:
                    # Y = X * tgt_b  (per-slot columns, per-partition scalar)
                    for s in range(sa, sb):
                        a = int(offs[s] - toff)
                        b = int(offs[s + 1] - toff)
                        nc.gpsimd.tensor_scalar_mul(
                            y[:, a:b], x[:, a:b].bitcast(F32), tgt[:, s : s + 1]
                        )

                ntiles = (tlen + TILE_N - 1) // TILE_N
                for j in range(ntiles):
                    c0 = j * TILE_N
                    c1 = min(tlen, c0 + TILE_N)
                    n = c1 - c0
                    # slot segments covered by this tile (chunk-local cols)
                    segs = []
                    for s in range(sa, sb):
                        a = max(int(offs[s] - toff), c0)
                        b = min(int(offs[s + 1] - toff), c1)
                        if a < b:
                            segs.append((s, a, b))

                    zz = []
                    for h in (0, 1):
                        z = pspool.tile([D, TILE_N], F32, tag=f"z{h}")
                        hs = slice(h * D, h * D + D)
                        nc.tensor.matmul(
                            z[:, :n],
                            wbc[:, hs],
                            x[:, c0:c1],
                            start=True,
                            stop=False,
                        )
                        if HOST_Y_BF16:
                            nc.tensor.matmul(
                                z[:, :n],
                                wd[:, hs],
                                y[:, c0:c1],
                                start=False,
                                stop=False,
                            )
                        else:
                            nc.tensor.matmul(
                                z[:, :n],
                                wd[:, hs].bitcast(F32R),
                                y[:, c0:c1].bitcast(F32R),
                                start=False,
                                stop=False,
                            )
                        # per-slot bias via 32-slot window indicator matmul
                        wins = {}
                        for s, a, b in segs:
                            w = s // 32
                            if w in wins:
                                lo, hi = wins[w]
                                wins[w] = (min(lo, a), max(hi, b))
                            else:
                                wins[w] = (a, b)
                        witems = sorted(wins.items())
                        for wi, (w, (a, b)) in enumerate(witems):
                            nc.tensor.matmul(
                                z[:, a - c0 : b - c0],
                                cbw[
                                    :, w * HID + h * D : w * HID + h * D + D
                                ],
                                indt[:, a:b],
                                start=False,
                                stop=(wi == len(witems) - 1),
                            )
                        zz.append(z)

                    r0 = rpool.tile([D, TILE_N], RDT, tag="r0")
                    r1 = rpool.tile([D, TILE_N], RDT, tag="r1")
                    nc.scalar.activation(
                        r0[:, :n], zz[0][:, :n], mybir.ActivationFunctionType.Relu
                    )
                    nc.scalar.activation(
                        r1[:, :n], zz[1][:, :n], mybir.ActivationFunctionType.Relu
                    )

                    # P[:, t] = score(t) + b2 on every partition
                    pbc = pspool.tile([D, TILE_N], F32, tag="pbc")
                    if RELU_BF16:
                        w2r0, w2r1 = w2r[:, 0:D], w2r[:, D:HID]
                        rr0, rr1 = r0[:, :n], r1[:, :n]
                    else:
                        w2r0 = w2r[:, 0:D].bitcast(F32R)
                        w2r1 = w2r[:, D:HID].bitcast(F32R)
                        rr0 = r0[:, :n].bitcast(F32R)
                        rr1 = r1[:, :n].bitcast(F32R)
                    nc.tensor.matmul(pbc[:, :n], w2r0, rr0, start=True, stop=False)
                    nc.tensor.matmul(pbc[:, :n], w2r1, rr1, start=False, stop=True)

                    dump = dpool.tile([D, TILE_N], F32, tag="dump")
                    if REDUCE_MODE == "ttr":
                        for s, a, b in segs:
                            first = a == int(offs[s] - toff)
                            nc.vector.tensor_tensor_reduce(
                                out=dump[:, a - c0 : b - c0],
                                in0=(x[:, a:b] if XB_BF16 else x[:, a:b].bitcast(F32)),
                                in1=pbc[:, a - c0 : b - c0],
                                scale=1.0,
                                scalar=0.0 if first else acc[:, s : s + 1],
                                op0=mybir.AluOpType.mult,
                                op1=mybir.AluOpType.add,
                                accum_out=acc[:, s : s + 1],
                            )
                    elif REDUCE_MODE == "stt":
                        for s, a, b in segs:
                            first = a == int(offs[s] - toff)
                            tgt_col = (
                                acc[:, s : s + 1]
                                if first
                                else aux[:, 0:1]
                            )
                            nc.vector.scalar_tensor_tensor(
                                out=dump[:, a - c0 : b - c0],
                                in0=pbc[:, a - c0 : b - c0],
                                scalar=B2VAL[0],
                                in1=(
                                    x[:, a:b]
                                    if XB_BF16
                                    else x[:, a:b].bitcast(F32)
                                ),
                                op0=mybir.AluOpType.add,
                                op1=mybir.AluOpType.mult,
                                accum_out=tgt_col,
                            )
                            if not first:
                                nc.vector.tensor_add(
                                    acc[:, s : s + 1],
                                    acc[:, s : s + 1],
                                    aux[:, 0:1],
                                )
                    else:  # "ttred"
                        nc.vector.tensor_tensor(
                            out=dump[:, :n],
                            in0=(x[:, c0:c1] if XB_BF16 else x[:, c0:c1].bitcast(F32)),
                            in1=pbc[:, :n],
                            op=mybir.AluOpType.mult,
                        )
                        for s, a, b in segs:
                            first = a == int(offs[s] - toff)
                            tgt_col = (
                                acc[:, s : s + 1] if first else aux[:, 0:1]
                            )
                            nc.vector.tensor_reduce(
                                out=tgt_col,
                                in_=dump[:, a - c0 : b - c0],
                                axis=mybir.AxisListType.X,
                                op=mybir.AluOpType.add,
                            )
                            if not first:
                                nc.vector.tensor_add(
                                    acc[:, s : s + 1],
                                    acc[:, s : s + 1],
                                    aux[:, 0:1],
                                )

            nc.sync.dma_start(out=out_d[:], in_=acc[:])
    nc.compile()
    return nc


def _pack_core(item_seq, target, cmat, nvec, order, slot_lens, offs, T, core):
    S = len(slot_lens)
    NW = (S + 31) // 32
    x_nat = np.zeros((T, D), dtype=np.float32)
    y_nat = np.zeros((T, D), dtype=np.float32) if HOST_Y_BF16 else None
    from ml_dtypes import bfloat16

    ind = np.zeros((32, T), dtype=bfloat16)
    tgt = np.zeros((D, S), dtype=np.float32)
    cbw = np.zeros((32, NW * HID), dtype=bfloat16)
    for s in range(S):
        b = int(order[N_CORES * s + core])
        o = int(offs[s])
        nb = int(nvec[b])
        if nb > 0:
            x_nat[o : o + nb] = item_seq[b, :nb]
            if y_nat is not None:
                y_nat[o : o + nb] = item_seq[b, :nb] * target[b]
        ind[s % 32, o : o + slot_lens[s]] = 1.0
        tgt[:, s] = target[b]
        cbw[s % 32, (s // 32) * HID : (s // 32 + 1) * HID] = cmat[b]
    xt = np.ascontiguousarray(x_nat.T)
    if XB_BF16:
        from ml_dtypes import bfloat16

        xt = xt.astype(bfloat16)
    m = {"xt": xt, "ind": ind, "cbw": cbw}
    if HOST_Y_BF16:
        from ml_dtypes import bfloat16

        m["yb"] = np.ascontiguousarray(y_nat.T).astype(bfloat16)
    else:
        m["tgt"] = tgt
    return m


def build_all(target, item_seq, seq_len, W1, b1, W2, b2):
    """Build (nc, in_maps, assemble) without running — used by kernel()
    and by test harnesses that want to run/profile the program."""
    target = np.asarray(target, dtype=np.float32)
    item_seq = np.asarray(item_seq, dtype=np.float32)
    W1 = np.asarray(W1, dtype=np.float32)
    b1 = np.asarray(b1, dtype=np.float32)
    W2 = np.asarray(W2, dtype=np.float32)
    b2 = np.asarray(b2, dtype=np.float32)

    nvec, order, slot_lens, offs, T, chunks = _plan(seq_len)
    S = len(slot_lens)

    W1a, W1b = W1[0:D], W1[D : 2 * D]
    W1c, W1d = W1[2 * D : 3 * D], W1[3 * D : 4 * D]
    wbc = np.ascontiguousarray(W1b + W1c)
    wd = np.ascontiguousarray(W1d)
    cmat = (target @ (W1a - W1c) + b1).astype(np.float32)  # (B, 256)
    w2r = np.empty((D, HID), dtype=np.float32)
    w2r[:, 0:D] = np.repeat(W2[0:D, 0:1], D, axis=1)  # [k, m] = W2[k]
    w2r[:, D:HID] = np.repeat(W2[D:HID, 0:1], D, axis=1)
    B2VAL[0] = float(np.asarray(b2).reshape(-1)[0])

    if HOST_Y_BF16 or RELU_BF16:
        from ml_dtypes import bfloat16
    if HOST_Y_BF16:
        wd = wd.astype(bfloat16)
    if XB_BF16:
        wbc = wbc.astype(bfloat16)
    if RELU_BF16:
        w2r = w2r.astype(bfloat16)

    nc = _build_program(slot_lens, offs, T, chunks)

    shared = {"wbc": wbc, "wd": wd, "w2r": w2r}
    in_maps = []
    for k in range(N_CORES):
        m = _pack_core(item_seq, target, cmat, nvec, order, slot_lens, offs, T, k)
        m.update(shared)
        in_maps.append(m)

    def assemble(results):
        out = np.zeros((B_FULL, D), dtype=np.float32)
        for k in range(N_CORES):
            ot = np.asarray(results[k]["out_t"])  # (128, 256)
            for s in range(S):
                out[int(order[N_CORES * s + k])] = ot[:, s]
        return out

    return nc, in_maps, assemble


def kernel(target, item_seq, seq_len, W1, b1, W2, b2):
    nc, in_maps, assemble = build_all(target, item_seq, seq_len, W1, b1, W2, b2)
    res = run_bass_kernel_spmd(nc, in_maps, list(range(N_CORES)))
    results = res.results if hasattr(res, "results") else res
    return assemble(results)



# revision 8
# speedup vs baseline: 1.1385x; 1.1385x over previous
"""DIN-attention kernel for Trainium2, 8-core SPMD.

Reference computation (per batch b, seq pos l, x = item_seq[b, l]):
    mlp_in = [tgt, x, x-tgt, x*tgt]           (4D = 512)
    h      = relu(mlp_in @ W1 + b1)           (2D = 256)
    score  = h @ W2 + b2                      (1)
    out_b  = sum_l score[l] * x[l] * (l < seq_len[b])

Algebraic restructure (W1 = [A; B; C; Dm] in 128-row blocks):
    z   = x @ (B + C) + (x*tgt) @ Dm + c_b,   c_b = tgt_b @ (A - C) + b1
    out = sum_{l < n_b} (W2.T relu(z) + b2) * x[l]

Device strategy (per core):
  - Batches sorted by seq_len descending; slot s holds global ranks
    [8s, 8s+8), one per core, padded to a shared per-slot length L_s
    (max over the 8, rounded even).  Zero-padded columns contribute 0,
    so all 8 cores run an identical (SPMD) program while loading only
    ~half the dense bytes.
  - Tokens packed host-side transposed (128=D, T).  Tiles are
    SLOT-ALIGNED: each PSUM tile covers whole slots (<= 512 cols) and
    never crosses a 32-slot bias window, so per tile there are exactly
    2 bias matmuls and per slot exactly one fused reduce op
    (accum_out writes the output column directly; no chaining).
  - hidden-on-partitions: z half h = Wbc_h.T X + Wd_h.T Y + Cwin_h.T IND
    in one PSUM accumulation group; relu on ScalarE (f32r out);
    P = W2rep0.T r0 + W2rep1.T r1 broadcasts score to 128 partitions;
    DVE scalar_tensor_tensor((P + b2) * X) with accum_out -> acc[:, s].
  - Chunks taper: small first chunks hide the DMA pipeline fill, a
    small last chunk shrinks the post-matmul reduce tail.
"""

import sys

import numpy as np

for _p in ("/opt/trn_rl_repo",):
    if _p not in sys.path:
        sys.path.insert(0, _p)

import concourse.bacc as bacc
import concourse.bass as bass
import concourse.tile as tile
from concourse import mybir
from concourse.bass_utils import run_bass_kernel_spmd

assert bass  # re-exported for callers

B_FULL = 2048
L_FULL = 200
D = 128
N_CORES = 8
HID = 256  # 2D
TILE_N = 512  # fp32 PSUM bank columns
F32 = mybir.dt.float32
F32R = mybir.dt.float32r
BF16 = mybir.dt.bfloat16

WIN = 32  # slots per bias window
B2VAL = [0.0]  # b2 constant, set by build_all before tracing


def _plan(seq_len):
    """Slot / tile / chunk plan shared by all cores (SPMD)."""
    n = np.clip(np.asarray(seq_len).astype(np.int64), 0, L_FULL)
    order = np.argsort(-n, kind="stable")  # descending
    n_sorted = n[order]
    rank_lens = []
    for s in range(B_FULL // N_CORES):
        m = int(n_sorted[N_CORES * s])  # max of ranks [8s, 8s+8)
        if m <= 0:
            break
        rank_lens.append(m + (m & 1))  # round up to even
    S = len(rank_lens)
    # Interleave long and short slots in the stream so the per-tile count
    # of reduce ops (one per slot) stays uniform: the reduce engine never
    # faces a burst of tiny slots at the end of the stream.
    perm = []
    lo, hi = 0, S - 1
    while lo <= hi:
        perm.append(lo)
        if hi != lo:
            perm.append(hi)
        lo += 1
        hi -= 1
    perm = np.asarray(perm, dtype=np.int64)  # stream slot j holds rank-group perm[j]
    slot_lens = [rank_lens[int(p)] for p in perm]
    offs = np.zeros(S + 1, dtype=np.int64)
    offs[1:] = np.cumsum(slot_lens)
    T = int(offs[-1])

    # Slot-aligned tiles: whole slots, <= TILE_N tokens, never crossing a
    # WIN-slot window boundary.
    tiles = []  # (slot_a, slot_b)  [a, b) slots
    sa = 0
    while sa < S:
        sb = sa
        wend = (sa // WIN + 1) * WIN
        while (
            sb < S
            and sb < wend
            and offs[sb + 1] - offs[sa] <= TILE_N
        ):
            sb += 1
        if sb == sa:
            sb = sa + 1
        tiles.append((sa, sb))
        sa = sb

    # Chunks: groups of whole tiles with tapered token budgets.
    budgets = [512, 1024, 2048, 4096]
    chunks = []  # (tile_a, tile_b, tok_off, tok_len)
    ta = 0
    bi = 0
    while ta < len(tiles):
        rem = T - int(offs[tiles[ta][0]])
        if bi < len(budgets):
            cap = budgets[bi]
        elif rem > 9216:
            cap = 8192
        else:
            # split the remainder so the last chunk is small
            cap = max(1024, rem - 1024)
        bi += 1
        tb = ta
        start = int(offs[tiles[ta][0]])
        while tb < len(tiles) and int(offs[tiles[tb][1]]) - start <= cap:
            tb += 1
        if tb == ta:
            tb = ta + 1
        end = int(offs[tiles[tb - 1][1]])
        chunks.append((ta, tb, start, end - start))
        ta = tb
    return n, order, perm, slot_lens, offs, T, tiles, chunks


def _build_program(slot_lens, offs, T, tiles, chunks):
    S = len(slot_lens)
    NW = (S + WIN - 1) // WIN
    nc = bacc.Bacc("TRN2", target_bir_lowering=False, debug=False)

    xy_d = nc.dram_tensor("xy", [D, 2 * T], BF16, kind="ExternalInput")
    ind_d = nc.dram_tensor("ind", [WIN, T], BF16, kind="ExternalInput")
    cbw_d = nc.dram_tensor("cbw", [WIN, NW * HID], BF16, kind="ExternalInput")
    wbc_d = nc.dram_tensor("wbc", [D, HID], BF16, kind="ExternalInput")
    wd_d = nc.dram_tensor("wd", [D, HID], BF16, kind="ExternalInput")
    w2r_d = nc.dram_tensor("w2r", [D, HID], F32R, kind="ExternalInput")
    out_d = nc.dram_tensor("out_t", [D, 256], F32, kind="ExternalOutput")

    cmax = max(c[3] for c in chunks)

    with tile.TileContext(nc) as tc:
        with (
            tc.tile_pool(name="const", bufs=1) as cpool,
            tc.tile_pool(name="xst", bufs=3) as xpool,
            tc.tile_pool(name="ist", bufs=3) as ipool,
            tc.tile_pool(name="rst", bufs=4) as rpool,
            tc.tile_pool(name="dst", bufs=4) as dpool,
            tc.tile_pool(name="zps", bufs=2, space="PSUM") as zpool,
            tc.tile_pool(name="pps", bufs=4, space="PSUM") as ppool,
        ):
            wbc = cpool.tile([D, HID], BF16, tag="wbc")
            wd = cpool.tile([D, HID], BF16, tag="wd")
            w2r = cpool.tile([D, HID], F32R, tag="w2r")
            cbw = cpool.tile([WIN, NW * HID], BF16, tag="cbw")
            acc = cpool.tile([D, 256], F32, tag="acc")

            nc.vector.memset(acc[:], 0.0)
            first = True

            for ta, tb, toff, tlen in chunks:
                xy = xpool.tile([D, 2 * cmax], BF16, tag="xy")
                indt = ipool.tile([WIN, cmax], BF16, tag="ind")
                nc.sync.dma_start(
                    out=xy[:, : 2 * tlen], in_=xy_d[:, 2 * toff : 2 * toff + 2 * tlen]
                )
                nc.sync.dma_start(out=indt[:, :tlen], in_=ind_d[:, toff : toff + tlen])
                if first:
                    # weights land after the first chunk's stream data so the
                    # PE pipeline fills as early as possible
                    nc.sync.dma_start(out=wbc[:], in_=wbc_d[:])
                    nc.sync.dma_start(out=wd[:], in_=wd_d[:])
                    nc.sync.dma_start(out=cbw[:], in_=cbw_d[:])
                    nc.sync.dma_start(out=w2r[:], in_=w2r_d[:])
                    first = False
                x = xy[:, 0 : 2 * cmax : 2]
                y = xy[:, 1 : 2 * cmax : 2]

                for ti in range(ta, tb):
                    sa, sb = tiles[ti]
                    c0 = int(offs[sa] - toff)
                    c1 = int(offs[sb] - toff)
                    nn = c1 - c0
                    w = sa // WIN  # single window per tile by construction

                    zz = []
                    for h in (0, 1):
                        z = zpool.tile([D, TILE_N], F32, tag=f"z{h}")
                        hs = slice(h * D, h * D + D)
                        nc.tensor.matmul(
                            z[:, :nn], wbc[:, hs], x[:, c0:c1],
                            start=True, stop=False,
                        )
                        nc.tensor.matmul(
                            z[:, :nn], wd[:, hs], y[:, c0:c1],
                            start=False, stop=False,
                        )
                        nc.tensor.matmul(
                            z[:, :nn],
                            cbw[:, w * HID + h * D : w * HID + h * D + D],
                            indt[:, c0:c1],
                            start=False, stop=True,
                        )
                        zz.append(z)

                    r0 = rpool.tile([D, TILE_N], F32R, tag="r0")
                    r1 = rpool.tile([D, TILE_N], F32R, tag="r1")
                    nc.scalar.activation(
                        r0[:, :nn], zz[0][:, :nn], mybir.ActivationFunctionType.Relu
                    )
                    nc.scalar.activation(
                        r1[:, :nn], zz[1][:, :nn], mybir.ActivationFunctionType.Relu
                    )

                    # P[:, t] = score(t) on every partition
                    pbc = ppool.tile([D, TILE_N], F32, tag="pbc")
                    nc.tensor.matmul(
                        pbc[:, :nn], w2r[:, 0:D],
                        r0[:, :nn], start=True, stop=False,
                    )
                    nc.tensor.matmul(
                        pbc[:, :nn], w2r[:, D:HID],
                        r1[:, :nn], start=False, stop=True,
                    )

                    dump = dpool.tile([D, TILE_N], F32, tag="dump")
                    for s in range(sa, sb):
                        a = int(offs[s] - toff)
                        b = int(offs[s + 1] - toff)
                        nc.vector.scalar_tensor_tensor(
                            out=dump[:, a - c0 : b - c0],
                            in0=pbc[:, a - c0 : b - c0],
                            scalar=B2VAL[0],
                            in1=x[:, a:b],
                            op0=mybir.AluOpType.add,
                            op1=mybir.AluOpType.mult,
                            accum_out=acc[:, s : s + 1],
                        )

            nc.sync.dma_start(out=out_d[:], in_=acc[:])
    nc.compile()
    return nc


def _pack_core(item_seq, target, cmat, nvec, order, perm, slot_lens, offs, T, core):
    from ml_dtypes import bfloat16

    S = len(slot_lens)
    NW = (S + WIN - 1) // WIN
    x_nat = np.zeros((T, D), dtype=np.float32)
    y_nat = np.zeros((T, D), dtype=np.float32)
    ind = np.zeros((WIN, T), dtype=bfloat16)
    cbw = np.zeros((WIN, NW * HID), dtype=bfloat16)
    for s in range(S):
        b = int(order[N_CORES * int(perm[s]) + core])
        o = int(offs[s])
        nb = int(nvec[b])
        if nb > 0:
            x_nat[o : o + nb] = item_seq[b, :nb]
            y_nat[o : o + nb] = item_seq[b, :nb] * target[b]
        ind[s % WIN, o : o + slot_lens[s]] = 1.0
        cbw[s % WIN, (s // WIN) * HID : (s // WIN + 1) * HID] = cmat[b]
    xy = np.empty((D, 2 * T), dtype=bfloat16)
    xy[:, 0::2] = x_nat.T
    xy[:, 1::2] = y_nat.T
    return {"xy": xy, "ind": ind, "cbw": cbw}


def build_all(target, item_seq, seq_len, W1, b1, W2, b2):
    """Build (nc, in_maps, assemble) without running — used by kernel()
    and by test harnesses that want to run/profile the program."""
    from ml_dtypes import bfloat16

    target = np.asarray(target, dtype=np.float32)
    item_seq = np.asarray(item_seq, dtype=np.float32)
    W1 = np.asarray(W1, dtype=np.float32)
    b1 = np.asarray(b1, dtype=np.float32)
    W2 = np.asarray(W2, dtype=np.float32)
    b2 = np.asarray(b2, dtype=np.float32)

    nvec, order, perm, slot_lens, offs, T, tiles, chunks = _plan(seq_len)

    W1a, W1b = W1[0:D], W1[D : 2 * D]
    W1c, W1d = W1[2 * D : 3 * D], W1[3 * D : 4 * D]
    wbc = np.ascontiguousarray(W1b + W1c).astype(bfloat16)
    wd = np.ascontiguousarray(W1d).astype(bfloat16)
    cmat = (target @ (W1a - W1c) + b1).astype(np.float32)  # (B, 256)
    w2r = np.empty((D, HID), dtype=np.float32)
    w2r[:, 0:D] = np.repeat(W2[0:D, 0:1], D, axis=1)  # [k, m] = W2[k]
    w2r[:, D:HID] = np.repeat(W2[D:HID, 0:1], D, axis=1)
    B2VAL[0] = float(np.asarray(b2).reshape(-1)[0])

    nc = _build_program(slot_lens, offs, T, tiles, chunks)

    shared = {"wbc": wbc, "wd": wd, "w2r": w2r}
    in_maps = []
    for k in range(N_CORES):
        m = _pack_core(item_seq, target, cmat, nvec, order, perm, slot_lens, offs, T, k)
        m.update(shared)
        in_maps.append(m)

    S = len(slot_lens)

    def assemble(results):
        out = np.zeros((B_FULL, D), dtype=np.float32)
        for k in range(N_CORES):
            ot = np.asarray(results[k]["out_t"])  # (128, 256)
            for s in range(S):
                out[int(order[N_CORES * int(perm[s]) + k])] = ot[:, s]
        return out

    return nc, in_maps, assemble


def kernel(target, item_seq, seq_len, W1, b1, W2, b2):
    nc, in_maps, assemble = build_all(target, item_seq, seq_len, W1, b1, W2, b2)
    res = run_bass_kernel_spmd(nc, in_maps, list(range(N_CORES)))
    results = res.results if hasattr(res, "results") else res
    return assemble(results)


# revision 39
# speedup vs baseline: 1.3890x; 1.2200x over previous
"""DIN-attention kernel for Trainium2, 8-core SPMD.

Reference computation (per batch b, seq pos l, x = item_seq[b, l]):
    mlp_in = [tgt, x, x-tgt, x*tgt]           (4D = 512)
    h      = relu(mlp_in @ W1 + b1)           (2D = 256)
    score  = h @ W2 + b2                      (1)
    out_b  = sum_l score[l] * x[l] * (l < seq_len[b])

Algebraic restructure (W1 = [A; B; C; Dm] in 128-row blocks):
    z   = x @ Wx + y @ Wy + c_b,   Wx = B + C, Wy = Dm, y = x * tgt_b,
    c_b = tgt_b @ (A - C) + b1
    out = sum_{l < n_b} (W2.T relu(z) + b2) * x[l]

Device strategy (per core):
  - Batches sorted by seq_len descending; slot s holds global ranks
    [8s, 8s+8), one per core, padded to a shared per-slot length (max of
    the 8, rounded even); zero-padded columns contribute 0, so all 8
    cores run one identical (SPMD) program.  Slots are then interleaved
    long/short in the token stream so the per-tile count of reduce ops
    stays uniform (no burst of tiny slots at the end).
  - fp8 DoubleRow z-matmuls at bf16-grade accuracy: the host ships a
    quad stream (x8, rx, y8, ry) = (fp8(x), fp8(x - x8), fp8(y),
    fp8(y - y8)) interleaved per token, plus weights
    W8 = fp8(W), E16 = fp8(16 (W - W8)).  Per hidden half, PSUM gets
      16 z = (16W8x, 16W8x).(x8, rx) + (16W8y, 16W8y).(y8, ry)
           + (E16x*16, E16y*16).(x8, y8) + (16 cbw).T ind
    i.e. 3 K=256 DoubleRow matmuls (0.5 cyc/col) + 1 bias matmul on the
    32-slot window one-hot; relu on ScalarE applies scale 1/16.
  - Tiles are slot-aligned (whole slots, <= 512 cols, never crossing a
    bias window) so each slot's reduce is ONE fused DVE op:
    tensor_tensor_reduce(x * P) with accum_out -> acc[:, s], where
    P = W2rep0.T r0 + W2rep1.T r1 broadcasts the score to all rows
    (b2 is folded in by the reduce op's scalar operand).
  - Software pipelining: tile i's score matmuls + reduces are emitted
    after tile i+1's z matmuls, giving every relu a full tile of slack.
  - A separate bf16 x stream feeds the reduce at full precision; chunk
    sizes taper up (1280, 2560, then ~4K) to hide the DMA fill, and the
    final per-slot column leaves the device already complete.
"""

import sys

import numpy as np

for _p in ("/opt/trn_rl_repo",):
    if _p not in sys.path:
        sys.path.insert(0, _p)

import concourse.bacc as bacc
import concourse.bass as bass
import concourse.tile as tile
from concourse import mybir
from concourse.bass_utils import run_bass_kernel_spmd

assert bass  # re-exported for callers

B_FULL = 2048
L_FULL = 200
D = 128
N_CORES = 8
HID = 256  # 2D
TILE_N = 512  # fp32 PSUM bank columns
F32 = mybir.dt.float32
F32R = mybir.dt.float32r
BF16 = mybir.dt.bfloat16
FP8 = mybir.dt.float8e4
DRMODE = mybir.MatmulPerfMode.DoubleRow

WIN = 64  # slots per bias window (one-hot padded to 128 partitions)
B2VAL = [0.0]  # b2 constant, set by build_all before tracing


def _plan(seq_len):
    """Slot / tile / chunk plan shared by all cores (SPMD)."""
    n = np.clip(np.asarray(seq_len).astype(np.int64), 0, L_FULL)
    order = np.argsort(-n, kind="stable")  # descending
    n_sorted = n[order]
    rank_lens = []
    for s in range(B_FULL // N_CORES):
        m = int(n_sorted[N_CORES * s])  # max of ranks [8s, 8s+8)
        if m <= 0:
            break
        rank_lens.append(m + (m & 1))  # round up to even
    S = len(rank_lens)
    # Interleave long and short slots in the stream so the per-tile count
    # of reduce ops (one per slot) stays uniform: the reduce engine never
    # faces a burst of tiny slots at the end of the stream.
    perm = []
    lo, hi = 0, S - 1
    while lo <= hi:
        perm.append(lo)
        if hi != lo:
            perm.append(hi)
        lo += 1
        hi -= 1
    perm = np.asarray(perm, dtype=np.int64)  # stream slot j holds rank-group perm[j]
    slot_lens = [rank_lens[int(p)] for p in perm]
    offs = np.zeros(S + 1, dtype=np.int64)
    offs[1:] = np.cumsum(slot_lens)
    T = int(offs[-1])

    # Slot-aligned tiles: whole slots, <= TILE_N tokens, never crossing a
    # WIN-slot window boundary.
    tiles = []  # (slot_a, slot_b)  [a, b) slots
    sa = 0
    while sa < S:
        sb = sa
        wend = (sa // WIN + 1) * WIN
        while (
            sb < S
            and sb < wend
            and offs[sb + 1] - offs[sa] <= TILE_N
        ):
            sb += 1
        if sb == sa:
            sb = sa + 1
        tiles.append((sa, sb))
        sa = sb

    # Chunks: groups of whole tiles with tapered token budgets.
    budgets = [1280, 2560, 4096, 3072, 4096, 3072]
    chunks = []  # (tile_a, tile_b, tok_off, tok_len)
    ta = 0
    bi = 0
    while ta < len(tiles):
        rem = T - int(offs[tiles[ta][0]])
        if bi < len(budgets):
            cap = budgets[bi]
        elif rem > 5120:
            cap = 4096
        else:
            # split the remainder so the last chunk is small
            cap = max(1536, rem - 1536)
        bi += 1
        tb = ta
        start = int(offs[tiles[ta][0]])
        while tb < len(tiles) and int(offs[tiles[tb][1]]) - start <= cap:
            tb += 1
        if tb == ta:
            tb = ta + 1
        end = int(offs[tiles[tb - 1][1]])
        chunks.append((ta, tb, start, end - start))
        ta = tb
    return n, order, perm, slot_lens, offs, T, tiles, chunks


def _build_program(slot_lens, offs, T, tiles, chunks):
    S = len(slot_lens)
    NW = (S + WIN - 1) // WIN
    nc = bacc.Bacc("TRN2", target_bir_lowering=False, debug=False)

    xq_d = nc.dram_tensor("xq", [D, 5 * T], FP8, kind="ExternalInput")
    xt_d = nc.dram_tensor("xt", [D, T], BF16, kind="ExternalInput")
    WQ2 = 4 * 2 * D + NW * 4 * 2 * D  # m1/m2 blocks + (w,h) pair blocks
    wq_d = nc.dram_tensor("wq", [D, WQ2], FP8, kind="ExternalInput")
    w2r_d = nc.dram_tensor("w2r", [D, HID], F32R, kind="ExternalInput")
    out_d = nc.dram_tensor("out_t", [D, 256], F32, kind="ExternalOutput")

    cmax = max(c[3] for c in chunks)

    with tile.TileContext(nc) as tc:
        with (
            tc.tile_pool(name="const", bufs=1) as cpool,
            tc.tile_pool(name="xst", bufs=4) as xpool,
            tc.tile_pool(name="qst", bufs=3) as qpool,
            tc.tile_pool(name="ist", bufs=3) as ipool,
            tc.tile_pool(name="rst", bufs=4) as rpool,
            tc.tile_pool(name="dst", bufs=4) as dpool,
            tc.tile_pool(name="zps", bufs=2, space="PSUM") as zpool,
            tc.tile_pool(name="pps", bufs=4, space="PSUM") as ppool,
        ):
            wq = cpool.tile([D, WQ2], FP8, tag="wq")
            w2r = cpool.tile([D, HID], F32R, tag="w2r")
            acc = cpool.tile([D, 256], F32, tag="acc")

            nc.vector.memset(acc[:], 0.0)
            first = True
            prev = None  # (sa, sb, c0, toff, r0, r1, x)

            def flush_prev():
                nonlocal prev
                if prev is None:
                    return
                psa, psb, pc0, ptoff, pr0, pr1, px, pnn = prev
                pbc = ppool.tile([D, TILE_N], F32, tag="pbc")
                nc.tensor.matmul(
                    pbc[:, :pnn], w2r[:, 0:D], pr0[:, :pnn],
                    start=True, stop=False,
                )
                nc.tensor.matmul(
                    pbc[:, :pnn], w2r[:, D:HID], pr1[:, :pnn],
                    start=False, stop=True,
                )
                dump = dpool.tile([D, TILE_N], F32, tag="dump")
                for s in range(psa, psb):
                    a = int(offs[s] - ptoff)
                    b = int(offs[s + 1] - ptoff)
                    nc.vector.scalar_tensor_tensor(
                        out=dump[:, a - pc0 : b - pc0],
                        in0=pbc[:, a - pc0 : b - pc0],
                        scalar=B2VAL[0],
                        in1=px[:, a:b],
                        op0=mybir.AluOpType.add,
                        op1=mybir.AluOpType.mult,
                        accum_out=acc[:, s : s + 1],
                    )
                prev = None

            for ta, tb, toff, tlen in chunks:
                xqt = xpool.tile([D, 5 * cmax], FP8, tag="xq")
                x = qpool.tile([D, cmax], BF16, tag="xt")
                if first:
                    # arrival order matches first-tile consumption order:
                    # wq+xq feed the z matmuls, then cbw+ind; xt/w2r are only
                    # needed one tile later (pipelined score/reduce)
                    nc.sync.dma_start(out=wq[:], in_=wq_d[:])
                nc.sync.dma_start(
                    out=xqt[:, : 5 * tlen], in_=xq_d[:, 5 * toff : 5 * toff + 5 * tlen]
                )
                nc.sync.dma_start(out=x[:, :tlen], in_=xt_d[:, toff : toff + tlen])
                if first:
                    nc.sync.dma_start(out=w2r[:], in_=w2r_d[:])
                    first = False
                qv = xqt[:].rearrange("p (n q) -> p q n", q=5)

                for ti in range(ta, tb):
                    sa, sb = tiles[ti]
                    c0 = int(offs[sa] - toff)
                    c1 = int(offs[sb] - toff)
                    nn = c1 - c0
                    w = sa // WIN  # single window per tile by construction

                    zz = []
                    for h in (0, 1):
                        z = zpool.tile([D, TILE_N], F32, tag=f"z{h}")

                        def wp(off):
                            return wq[:, off : off + 2 * D].rearrange(
                                "p (t m) -> p t m", t=2
                            )

                        pb = 8 * D + (w * 2 + h) * 4 * D
                        nc.tensor.matmul(
                            z[:, :nn], wp(h * 4 * D), qv[:, 0:2, c0:c1],
                            start=True, stop=False, perf_mode=DRMODE,
                        )
                        nc.tensor.matmul(
                            z[:, :nn], wp(h * 4 * D + 2 * D), qv[:, 2:4, c0:c1],
                            start=False, stop=False, perf_mode=DRMODE,
                        )
                        # (E16x | C8).(x8, ind) and (E16y | CR).(y8, ind): the
                        # weight residuals and the bias share the DoubleRows
                        nc.tensor.matmul(
                            z[:, :nn], wp(pb), qv[:, 0:5:4, c0:c1],
                            start=False, stop=False, perf_mode=DRMODE,
                        )
                        nc.tensor.matmul(
                            z[:, :nn], wp(pb + 2 * D), qv[:, 2:5:2, c0:c1],
                            start=False, stop=True, perf_mode=DRMODE,
                        )
                        zz.append(z)

                    r0 = rpool.tile([D, TILE_N], F32R, tag="r0")
                    r1 = rpool.tile([D, TILE_N], F32R, tag="r1")
                    nc.scalar.activation(
                        r0[:, :nn], zz[0][:, :nn],
                        mybir.ActivationFunctionType.Relu, scale=1.0 / 16.0,
                    )
                    nc.scalar.activation(
                        r1[:, :nn], zz[1][:, :nn],
                        mybir.ActivationFunctionType.Relu, scale=1.0 / 16.0,
                    )
                    # previous tile's score + reduce, emitted here so its relu
                    # had a full tile of slack (software pipelining)
                    flush_prev()
                    prev = (sa, sb, c0, toff, r0, r1, x, nn)

            flush_prev()
            nc.sync.dma_start(out=out_d[:], in_=acc[:])
    nc.compile()
    return nc


def _pack_core(item_seq, target, cmat, nvec, order, perm, slot_lens, offs, T, core):
    from ml_dtypes import bfloat16

    S = len(slot_lens)
    NW = (S + WIN - 1) // WIN
    x_nat = np.zeros((T, D), dtype=np.float32)
    y_nat = np.zeros((T, D), dtype=np.float32)
    from ml_dtypes import float8_e4m3

    ind = np.zeros((D, T), dtype=float8_e4m3)
    cstack = np.zeros((NW, D, HID), dtype=np.float32)  # slot-rows x hidden
    for s in range(S):
        b = int(order[N_CORES * int(perm[s]) + core])
        o = int(offs[s])
        nb = int(nvec[b])
        if nb > 0:
            x_nat[o : o + nb] = item_seq[b, :nb]
            y_nat[o : o + nb] = item_seq[b, :nb] * target[b]
        ind[s % WIN, o : o + slot_lens[s]] = 1.0
        cstack[s // WIN, s % WIN, :] = cmat[b]
    xT = np.ascontiguousarray(x_nat.T)
    yT = np.ascontiguousarray(y_nat.T)
    x8 = xT.astype(float8_e4m3)
    y8 = yT.astype(float8_e4m3)
    rx = (xT - x8.astype(np.float32)).astype(float8_e4m3)
    ry = (yT - y8.astype(np.float32)).astype(float8_e4m3)
    xq = np.empty((D, 5 * T), dtype=float8_e4m3)
    xq[:, 0::5], xq[:, 1::5], xq[:, 2::5] = x8, rx, y8
    xq[:, 3::5], xq[:, 4::5] = ry, ind
    return {"xq": xq, "xt": xT.astype(bfloat16), "cstack": cstack}


def build_all(target, item_seq, seq_len, W1, b1, W2, b2):
    """Build (nc, in_maps, assemble) without running — used by kernel()
    and by test harnesses that want to run/profile the program."""
    from ml_dtypes import bfloat16

    target = np.asarray(target, dtype=np.float32)
    item_seq = np.asarray(item_seq, dtype=np.float32)
    W1 = np.asarray(W1, dtype=np.float32)
    b1 = np.asarray(b1, dtype=np.float32)
    W2 = np.asarray(W2, dtype=np.float32)
    b2 = np.asarray(b2, dtype=np.float32)

    nvec, order, perm, slot_lens, offs, T, tiles, chunks = _plan(seq_len)

    from ml_dtypes import float8_e4m3

    def f8(a):
        return np.asarray(a, dtype=np.float32).astype(float8_e4m3)

    W1a, W1b = W1[0:D], W1[D : 2 * D]
    W1c, W1d = W1[2 * D : 3 * D], W1[3 * D : 4 * D]
    wbc = np.ascontiguousarray(W1b + W1c)  # x-side weights (128, 256)
    wd = np.ascontiguousarray(W1d)  # y-side weights
    w8x, w8y = f8(wbc), f8(wd)
    w16x = f8(16.0 * w8x.astype(np.float32))
    w16y = f8(16.0 * w8y.astype(np.float32))
    assert np.array_equal(w16x.astype(np.float32), 16.0 * w8x.astype(np.float32))
    assert np.array_equal(w16y.astype(np.float32), 16.0 * w8y.astype(np.float32))
    ex16 = f8(16.0 * (wbc - w8x.astype(np.float32)))
    ey16 = f8(16.0 * (wd - w8y.astype(np.float32)))
    gblocks = np.empty((D, 8 * D), dtype=float8_e4m3)
    for h in (0, 1):
        hs = slice(h * D, h * D + D)
        base = h * 4 * D
        gblocks[:, base + 0 * D : base + 1 * D] = w16x[:, hs]
        gblocks[:, base + 1 * D : base + 2 * D] = w16x[:, hs]
        gblocks[:, base + 2 * D : base + 3 * D] = w16y[:, hs]
        gblocks[:, base + 3 * D : base + 4 * D] = w16y[:, hs]
    cmat = (16.0 * (target @ (W1a - W1c) + b1)).astype(np.float32)  # (B, 256)
    w2r = np.empty((D, HID), dtype=np.float32)
    w2r[:, 0:D] = np.repeat(W2[0:D, 0:1], D, axis=1)  # [k, m] = W2[k]
    w2r[:, D:HID] = np.repeat(W2[D:HID, 0:1], D, axis=1)
    B2VAL[0] = float(np.asarray(b2).reshape(-1)[0])

    nc = _build_program(slot_lens, offs, T, tiles, chunks)

    from ml_dtypes import float8_e4m3 as _f8t

    shared = {"w2r": w2r}
    in_maps = []
    S = len(slot_lens)
    NW = (S + WIN - 1) // WIN
    for k in range(N_CORES):
        m = _pack_core(item_seq, target, cmat, nvec, order, perm, slot_lens, offs, T, k)
        m.update(shared)
        cstack = m.pop("cstack")  # (NW, D, HID), rows WIN..D are zero
        wqc = np.zeros((D, 8 * D + NW * 8 * D), dtype=_f8t)
        wqc[:, : 8 * D] = gblocks
        for w_ in range(NW):
            for h in (0, 1):
                cw = cstack[w_][:, h * D : h * D + D]
                c8 = cw.astype(_f8t)
                cr = (cw - c8.astype(np.float32)).astype(_f8t)
                pb = 8 * D + (w_ * 2 + h) * 4 * D
                wqc[:, pb + 0 * D : pb + 1 * D] = ex16[:, h * D : h * D + D]
                wqc[:, pb + 1 * D : pb + 2 * D] = c8
                wqc[:, pb + 2 * D : pb + 3 * D] = ey16[:, h * D : h * D + D]
                wqc[:, pb + 3 * D : pb + 4 * D] = cr
        m["wq"] = wqc
        in_maps.append(m)

    S = len(slot_lens)

    def assemble(results):
        out = np.zeros((B_FULL, D), dtype=np.float32)
        for k in range(N_CORES):
            ot = np.asarray(results[k]["out_t"])  # (128, 256)
            for s in range(S):
                out[int(order[N_CORES * int(perm[s]) + k])] = ot[:, s]
        return out

    return nc, in_maps, assemble


def kernel(target, item_seq, seq_len, W1, b1, W2, b2):
    nc, in_maps, assemble = build_all(target, item_seq, seq_len, W1, b1, W2, b2)
    res = run_bass_kernel_spmd(nc, in_maps, list(range(N_CORES)))
    results = res.results if hasattr(res, "results") else res
    return assemble(results)
